# revision 1
# baseline (speedup 1.0000x reference)
"""Trainium2 Bass kernel for a single-layer causal-attention decoder.

Model (per batch element):
    emb = emb_table[x]                      # [S, D] via one-hot matmul
    Q/K/V = emb @ w.T + b                   # folded: QV = emb_table @ w.T + b  [V, D]
    scores = Q @ K.T / sqrt(D), causal mask
    out = softmax(scores) @ V               # [S, D]

Sharding: data-parallel over batch. B=8 elements -> 8 NeuronCores, one
sequence per core; weights replicated. No collectives.

Device-side layout strategy (per core):
  - onehotT [V, S] from int32 x-broadcast vs device iota (is_equal).
  - QV/KV/VV [V, D(+1)] projected vocab tables computed on device; QT/KT
    [D, S] = QV.T @ onehotT feed the scores matmul directly (no transposes).
  - scoresT blocks [128k, 512q] = KT_chunk.T @ QT_chunk in bf16, only
    lower-triangular causal blocks, partial diagonal blocks narrowed to
    their valid column range, each block column-packed as two concurrent
    M=64 matmuls in different PE column groups.
  - expT = exp(scoresT * 0.125) on ACT straight out of PSUM (two k-blocks
    per instruction to amortize access latency); partial blocks go through a
    separate gpsimd affine_select into a masked tile (so every matmul keeps
    a single sync wait).
  - V_aug [S, D+1] with a ones column: outT accum += V_aug.T @ expT yields
    weighted values (d<64) and the softmax denominator (d=64) in one
    accumulation. Softmax max-subtraction is skipped: |scores| < ~6 here, so
    exp is safely inside fp32 range and normalization cancels it exactly.
  - epilogue: PE-transpose [65,128] chunks, multiply by reciprocal of the
    denominator column straight from PSUM, DMA out [S, D].
"""

import numpy as np

import concourse.bass as bass
import concourse.mybir as mybir
import concourse.tile as tile
from concourse import bacc
from concourse.bass_utils import run_bass_kernel_spmd

F32 = mybir.dt.float32
F32R = mybir.dt.float32r
I32 = mybir.dt.int32

B = 8
S = 2048
D = 64
V = 5
P = 128
QC = 512  # q-chunk (PSUM bank free-dim limit for fp32)
N_CORES = 8
CBW = V + 3 * (D + 1) + (D + 1) - 2 * 1  # 5 + 64 + 64 + 65 + 65 = 263

MODE = "bf16"  # "f32" | "f32r" | "bf16" | "hybrid"


def _body(tc, aps, S, mode):
    nc = tc.nc
    x, cb, out = aps["x"], aps["cb"], aps["out"]
    qc = min(QC, S)
    NK = S // P          # k-blocks
    NQ = S // qc         # q-chunks
    KPQ = qc // P        # k-blocks per q-chunk
    Exp = mybir.ActivationFunctionType.Exp

    # dtypes for the two matmul data paths. fp32r tensors must be written as
    # fp32r by their producer (walrus birverifier enforces the rounding);
    # gpsimd cannot write fp32r, but can write bf16.
    BF16 = mybir.dt.bfloat16
    DT_QK = {"f32": F32, "f32r": F32R, "bf16": BF16, "hybrid": F32R}[mode]
    DT_PV = {"f32": F32, "f32r": F32R, "bf16": BF16, "hybrid": BF16}[mode]

    from contextlib import ExitStack
    with ExitStack() as ctx:
        consts = ctx.enter_context(tc.tile_pool(name="consts", bufs=1))
        expp = ctx.enter_context(tc.tile_pool(name="expp", bufs=4))
        mep = ctx.enter_context(tc.tile_pool(name="mep", bufs=3))
        outp = ctx.enter_context(tc.tile_pool(name="outp", bufs=3))
        ps_small = ctx.enter_context(tc.tile_pool(name="ps_small", bufs=2, space="PSUM"))
        ps_att = ctx.enter_context(tc.tile_pool(name="ps_att", bufs=3, space="PSUM"))

        # ---- constants in (single DMA -> single wait downstream) ----
        cb_sb = consts.tile([D + 1, CBW], F32)
        nc.sync.dma_start(cb_sb[:], cb[:])
        etT = cb_sb[:, 0:V]                      # [D+1, V]  emb_table.T + ones row
        wqa = cb_sb[:, V : V + D]                # [D+1, D]
        wka = cb_sb[:, V + D : V + 2 * D]        # [D+1, D]
        wva = cb_sb[:, V + 2 * D : V + 3 * D + 1]      # [D+1, D+1]
        id65 = cb_sb[:, V + 3 * D + 1 : V + 4 * D + 2]  # [D+1, D+1]

        # ---- x broadcast + device iota ----
        xb = consts.tile([V, S], I32)
        nc.sync.dma_start(xb[:], x[None, :].to_broadcast((V, S)))
        io = consts.tile([V, 1], I32)
        nc.gpsimd.iota(io[:], pattern=[[0, 1]], base=0, channel_multiplier=1)

        # ---- per-vocab projected tables QV/KV [V, D], VV_aug [V, D+1] ----
        pqv = ps_small.tile([V, D], F32, tag="small")
        nc.tensor.matmul(pqv[:], lhsT=etT, rhs=wqa, start=True, stop=True)
        qv_sb = consts.tile([V, D], DT_QK)
        nc.vector.tensor_copy(qv_sb[:], pqv[:])

        pkv = ps_small.tile([V, D], F32, tag="small")
        nc.tensor.matmul(pkv[:], lhsT=etT, rhs=wka, start=True, stop=True)
        kv_sb = consts.tile([V, D], DT_QK)
        nc.vector.tensor_copy(kv_sb[:], pkv[:])

        # wv_aug has an extra column e_D: VV_aug = ET_aug @ wv_aug gets its
        # ones column from ET_aug's ones row
        pvv = ps_small.tile([V, D + 1], F32, tag="small")
        nc.tensor.matmul(pvv[:], lhsT=etT, rhs=wva, start=True, stop=True)
        # V matmul runs at DT_QK unless that's fp32r (odd moving-free dim 65
        # violates the fp32r ISA restriction) -> plain fp32 via bitcast then
        DT_V = F32 if DT_QK == F32R else DT_QK
        vv_sb = consts.tile([V, D + 1], DT_V)
        nc.vector.tensor_copy(vv_sb[:], pvv[:])

        # ---- one-hot, QT/KT [D, S], V_aug [P, NK, D+1], chunk-pipelined ----
        oh = consts.tile([V, S], DT_QK)
        qt_sb = consts.tile([D, S], DT_QK)
        kt_sb = consts.tile([D, S], DT_QK)
        vg_sb = consts.tile([P, NK, D + 1], DT_PV)
        def setup_chunk(j):
            sl = slice(j * qc, (j + 1) * qc)
            nc.vector.tensor_tensor(
                oh[:, sl], xb[:, sl], io[:].to_broadcast((V, qc)),
                mybir.AluOpType.is_equal,
            )
            pq = ps_small.tile([D, qc], F32, tag="small")
            nc.tensor.matmul(pq[:], lhsT=qv_sb[:], rhs=oh[:, sl], start=True, stop=True)
            nc.vector.tensor_copy(qt_sb[:, sl], pq[:])
            pk = ps_small.tile([D, qc], F32, tag="small")
            nc.tensor.matmul(pk[:], lhsT=kv_sb[:], rhs=oh[:, sl], start=True, stop=True)
            nc.vector.tensor_copy(kt_sb[:, sl], pk[:])
            for si in range(j * KPQ, (j + 1) * KPQ):
                pv = ps_small.tile([P, D + 1], F32, tag="small")
                lhs_v = oh[:, si * P : (si + 1) * P]
                if DT_QK == F32R:
                    lhs_v = lhs_v.bitcast(F32)
                nc.tensor.matmul(
                    pv[:], lhsT=lhs_v, rhs=vv_sb[:], start=True, stop=True,
                )
                nc.vector.tensor_copy(vg_sb[:, si, :], pv[:])

        # ---- causal attention, one q-chunk at a time ----
        # per k-block: scores matmul (narrowed to the causal-valid column
        # range) -> exp -> (diag: affine mask into a separate tile) -> PV
        # accumulate. PV for block ki is emitted after scores/exp of ki+2 so
        # the PE never stalls on the ACT/GpSimd stages; the epilogue of the
        # previous q-chunk is emitted inside the next chunk's stream for the
        # same reason.
        PVDEPTH = 2  # pairs of k-blocks in flight before their PV is emitted
        epilogue = [None]

        def emit_epilogue():
            if epilogue[0] is None:
                return
            po, qi = epilogue[0]
            epilogue[0] = None
            ot = outp.tile([D + 1, qc], F32, tag="ot")
            nc.vector.tensor_copy(ot[:], po[:])
            for j in range(KPQ):
                pt = ps_small.tile([P, D + 1], F32, tag="small")
                nc.tensor.transpose(pt[:], ot[:, j * P : (j + 1) * P], id65)
                ob = outp.tile([P, D + 1], F32, tag="ob")
                nc.vector.tensor_copy(ob[:], pt[:])
                rc = outp.tile([P, 1], F32, tag="rc")
                nc.vector.reciprocal(rc[:], ob[:, D : D + 1])
                rs_t = outp.tile([P, D], F32, tag="rs")
                nc.vector.tensor_mul(rs_t[:], ob[:, :D], rc[:].to_broadcast((P, D)))
                r0 = qi * qc + j * P
                nc.sync.dma_start(out[r0 : r0 + P, :], rs_t[:])

        def attention_chunk(qi):
            nki = (qi + 1) * KPQ
            po = ps_small.tile([D + 1, qc], F32, tag="small")

            def emit_pv(bundle, po=po, nki=nki):
                et, mes, kis, rs = bundle
                for h, ki in enumerate(kis):
                    r = rs[h]
                    if mes[h] is not None:
                        rhs = mes[h][:, r:qc]
                    else:
                        rhs = et[:, h * qc + r : (h + 1) * qc]
                    nc.tensor.matmul(
                        po[:, r:qc], lhsT=vg_sb[:, ki, :], rhs=rhs,
                        start=(ki == 0), stop=(ki == nki - 1),
                    )

            pending = []
            assert nki % 2 == 0
            for p2 in range(nki // 2):
                kis = [2 * p2, 2 * p2 + 1]
                rs = [max(0, ki * P - qi * qc) for ki in kis]
                ps = ps_att.tile([P, 2 * qc], F32, tag="att")
                et = expp.tile([P, 2 * qc], DT_PV, tag="exp")
                for h, ki in enumerate(kis):
                    r = rs[h]
                    # col-packed pair: two concurrent M=64 matmuls in
                    # different PE column groups (the k-block's two halves)
                    for cg in (0, 1):
                        nc.tensor.matmul(
                            ps[cg * D : (cg + 1) * D, h * qc + r : (h + 1) * qc],
                            lhsT=kt_sb[:, ki * P + cg * D : ki * P + (cg + 1) * D],
                            rhs=qt_sb[:, qi * qc + r : (qi + 1) * qc],
                            start=True, stop=True,
                            tile_position=(0, cg * D),
                        )
                if rs[1] > 0:
                    # narrowed halves: exp each half's valid window
                    for h in (0, 1):
                        r = rs[h]
                        nc.scalar.activation(
                            et[:, h * qc + r : (h + 1) * qc],
                            ps[:, h * qc + r : (h + 1) * qc], Exp, scale=0.125,
                        )
                else:
                    nc.scalar.activation(et[:], ps[:], Exp, scale=0.125)
                mes = [None, None]
                for h, ki in enumerate(kis):
                    if ki >= qi * KPQ:  # partially-masked diagonal block
                        r = rs[h]
                        me = mep.tile([P, qc], DT_PV, tag="me")
                        # window starts at col r; iota = j - kk, keep j >= kk
                        nc.gpsimd.affine_select(
                            out=me[:, r:qc], in_=et[:, h * qc + r : (h + 1) * qc],
                            pattern=[[1, qc - r]],
                            base=0,
                            channel_multiplier=-1,
                            compare_op=mybir.AluOpType.is_ge,
                            fill=0.0,
                        )
                        mes[h] = me
                if p2 == 0:
                    # previous q-chunk's epilogue rides behind this chunk's
                    # first scores matmuls
                    emit_epilogue()
                pending.append((et, mes, kis, rs))
                if len(pending) > PVDEPTH:
                    emit_pv(pending.pop(0))
            for b in pending:
                emit_pv(b)
            epilogue[0] = (po, qi)

        # emission schedule: attention(qi) only needs qt/kt chunks <= qi, so
        # it rides behind setup of chunk qi+1 and fills the PE while the DVE
        # builds the remaining projections
        for j in range(NQ):
            setup_chunk(j)
        for qi in range(NQ):
            attention_chunk(qi)
        emit_epilogue()


def build_nc(S=S, mode=MODE):
    # Bacc (not plain Bass): its compile() pass splits multi-waits off
    # matmuls — TRN2 fp32/fp32r self-loading matmuls only encode one wait
    nc = bacc.Bacc(trn_type="TRN2", target_bir_lowering=False, debug=False)
    aps = {}
    aps["x"] = nc.dram_tensor("x", [S], I32, kind="ExternalInput").ap()
    aps["cb"] = nc.dram_tensor("cb", [D + 1, CBW], F32, kind="ExternalInput").ap()
    aps["out"] = nc.dram_tensor("out", [S, D], F32, kind="ExternalOutput").ap()
    with tile.TileContext(nc) as tc:
        _body(tc, aps, S=S, mode=mode)
    nc.compile()
    return nc


def make_in_maps(x, emb_table, wq, bq, wk, bk, wv, bv, S=S, n_cores=N_CORES):
    x = np.asarray(x).astype(np.int32)
    emb_table = np.asarray(emb_table, dtype=np.float32)

    def aug(w, b):
        return np.vstack(
            [np.asarray(w, np.float32).T, np.asarray(b, np.float32)[None, :]]
        )  # [D+1, D]

    cbuf = np.zeros((D + 1, CBW), np.float32)
    cbuf[:, 0:V] = np.vstack([emb_table.T, np.ones((1, V), np.float32)])
    cbuf[:, V : V + D] = aug(wq, bq)
    cbuf[:, V + D : V + 2 * D] = aug(wk, bk)
    cbuf[:, V + 2 * D : V + 3 * D] = aug(wv, bv)
    cbuf[D, V + 3 * D] = 1.0  # e_D column of wv_aug -> ones column of VV_aug
    cbuf[:, V + 3 * D + 1 : V + 4 * D + 2] = np.eye(D + 1, dtype=np.float32)
    cbuf = np.ascontiguousarray(cbuf)

    return [
        dict(x=np.ascontiguousarray(x[c, :S]), cb=cbuf)
        for c in range(n_cores)
    ]


_NC_CACHE = {}


def _get_nc(S=S, mode=MODE):
    key = (S, mode)
    if key not in _NC_CACHE:
        _NC_CACHE[key] = build_nc(S=S, mode=mode)
    return _NC_CACHE[key]


def run(inputs, trace=False, **kw):
    in_maps = make_in_maps(**inputs)
    nc = _get_nc()
    res = run_bass_kernel_spmd(nc, in_maps, core_ids=list(range(N_CORES)), trace=trace, **kw)
    out = np.stack([res.results[c]["out"] for c in range(N_CORES)])
    return out, res


def kernel(x, emb_table, wq, bq, wk, bk, wv, bv):
    out, _ = run(dict(x=x, emb_table=emb_table, wq=wq, bq=bq, wk=wk, bk=bk,
                      wv=wv, bv=bv))
    return out



# revision 5
# speedup vs baseline: 2.9158x; 2.9158x over previous
"""Trainium2 Bass kernel for a single-layer causal-attention decoder.

Key algebraic shortcut: VOCAB=5 and the model has no positional encoding,
so Q[q], K[k], V[k] depend only on the token ids x_q, x_k. The [S, S]
attention therefore collapses to prefix token counts:

    E5[a, t]  = exp(Q5[a] . K5[t] / 8)          # [5, 5]
    C[t, q]   = #{k <= q : x_k = t}             # prefix counts
    out[q, :] = sum_t C[t,q] E5[x_q,t] V5[t,:] / sum_t C[t,q] E5[x_q,t]

which is O(S*V) work instead of O(S^2*D). No S x S matrices exist at all.

Device layout (per core; data-parallel over batch, one sequence per core):
  - positions packed 16 blocks x 128 on partitions: partition (b, t) = b*5+t
    holds token t's lane for position block b -> all elementwise work is
    [80, 128] instead of [5, 2048].
  - one-hot via is_equal(x_broadcast, t-column); in-block inclusive prefix
    count via the DVE tensor_tensor_scan; cross-block carries via one tiny
    matmul with a host-packed [80, 80] lower-block mask, folded into the
    weight multiply with scalar_tensor_tensor.
  - E5 selection by token is an [80, 80]-block-diag matmul (e5blk built
    with one masked multiply); W = C * E5[x_q, :].
  - output: po [128, 4, 65] = W.T @ R_g where R[(b,t), (bb,m)] =
    [b == bb] * V5aug[t, m] (built once on gpsimd) — 4 wide matmuls cover
    all 16 position blocks; the ones column of V5aug gives the softmax
    denominator; reciprocal + multiply + one DMA per group.
Softmax max-subtraction is skipped; |scores/8| < ~2 so exp is tame and the
normalization cancels it exactly.
"""

import numpy as np

import concourse.bass as bass
import concourse.mybir as mybir
import concourse.tile as tile
from concourse import bacc
from concourse.bass_utils import run_bass_kernel_spmd

F32 = mybir.dt.float32
BF16 = mybir.dt.bfloat16
I32 = mybir.dt.int32

B = 8
S = 2048
D = 64
V = 5
P = 128
N_CORES = 8
MODE = "bf16"


def _np(S):
    return V * (S // P)  # packed partitions: (block b, token t) -> b*V + t


def _cbw(S):
    # etT | wq | wk | wv_aug | io | Lmask | blkmask | hmask
    return V + 3 * D + 1 + 1 + 2 * _np(S) + (S // P)


def _body(tc, aps, S):
    nc = tc.nc
    x, cb, out = aps["x"], aps["cb"], aps["out"]
    KB = S // P          # position blocks (= 16 at S=2048)
    NP = V * KB          # packed partitions (= 80)
    CBP = max(D + 1, NP)
    Exp = mybir.ActivationFunctionType.Exp
    add = mybir.AluOpType.add
    mult = mybir.AluOpType.mult
    bypass = mybir.AluOpType.bypass
    is_equal = mybir.AluOpType.is_equal

    from contextlib import ExitStack
    with ExitStack() as ctx:
        consts = ctx.enter_context(tc.tile_pool(name="consts", bufs=1))
        outp = ctx.enter_context(tc.tile_pool(name="outp", bufs=2))
        ps_small = ctx.enter_context(tc.tile_pool(name="ps_small", bufs=2, space="PSUM"))
        ps_pg = ctx.enter_context(tc.tile_pool(name="ps_pg", bufs=1, space="PSUM"))
        ps_o = ctx.enter_context(tc.tile_pool(name="ps_o", bufs=2, space="PSUM"))

        # ---- constants in ----
        c0 = V + 3 * D + 1
        cb_sb = consts.tile([CBP, _cbw(S)], F32)
        nc.sync.dma_start(cb_sb[:], cb[:])
        etT = cb_sb[0 : D + 1, 0:V]                      # emb.T + ones row
        wqa = cb_sb[0 : D + 1, V : V + D]
        wka = cb_sb[0 : D + 1, V + D : V + 2 * D]
        wva = cb_sb[0 : D + 1, V + 2 * D : V + 3 * D + 1]
        io = cb_sb[0:NP, c0 : c0 + 1]                    # t = p % V
        lmask = cb_sb[0:NP, c0 + 1 : c0 + 1 + NP]
        bmask = cb_sb[0:NP, c0 + 1 + NP : c0 + 1 + 2 * NP]
        hmask = cb_sb[0:NP, c0 + 1 + 2 * NP : c0 + 1 + 2 * NP + KB]

        # ---- x packed+replicated on host: partition (b, t) holds x[b*128:(b+1)*128] ----
        xb = consts.tile([NP, P], I32)
        nc.sync.dma_start(xb[:], x[:])

        # ---- tiny projected tables ----
        # Q5T tiled KB times along columns: [D, NP], col (b,t) = Q5T[:, t]
        pq = ps_small.tile([D, V], F32, tag="sm")
        nc.tensor.matmul(pq[:], lhsT=wqa, rhs=etT, start=True, stop=True)
        q5t = consts.tile([D, NP], F32)
        nc.vector.tensor_copy(
            q5t[:].rearrange("d (b t) -> d b t", t=V),
            pq[:, None, :].to_broadcast((D, KB, V)),
        )
        pk = ps_small.tile([D, V], F32, tag="sm")
        nc.tensor.matmul(pk[:], lhsT=wka, rhs=etT, start=True, stop=True)
        k5t = consts.tile([D, V], F32)
        nc.vector.tensor_copy(k5t[:], pk[:])
        # etT tiled KB times along columns -> V5aug tiled on partitions
        etT80 = consts.tile([D + 1, NP], F32)
        nc.vector.tensor_copy(
            etT80[:].rearrange("d (b t) -> d b t", t=V),
            etT[:, None, :].to_broadcast((D + 1, KB, V)),
        )
        # V5aug80 [(b,t), m] = V5aug[t, m]; ones column from wva's e_D column
        pvv = ps_small.tile([NP, D + 1], F32, tag="sm")
        nc.tensor.matmul(pvv[:], lhsT=etT80[:], rhs=wva, start=True, stop=True)
        v5aug = consts.tile([NP, D + 1], BF16)
        nc.vector.tensor_copy(v5aug[:], pvv[:])

        # ---- R [(b,t), (bb, m)] = [b == bb] * V5aug[t, m] (gpsimd, off DVE) ----
        rsel = consts.tile([NP, KB, D + 1], BF16)
        nc.gpsimd.tensor_tensor(
            rsel[:],
            hmask[:, :, None].to_broadcast((NP, KB, D + 1)),
            v5aug[:, None, :].to_broadcast((NP, KB, D + 1)),
            mult,
        )

        # ---- one-hot + in-block prefix counts (DVE) ----
        oh = consts.tile([NP, P], BF16)
        nc.vector.tensor_tensor(oh[:], xb[:], io.to_broadcast((NP, P)), is_equal)
        cnt = consts.tile([NP, P], F32)
        nc.vector.tensor_tensor_scan(cnt[:], oh[:], oh[:], 0.0, add, bypass)

        # ---- cross-block carries: offs[(b,t)] = sum_{b'<b} total[(b',t)] ----
        poffs = ps_small.tile([NP, 1], F32, tag="sm")
        nc.tensor.matmul(poffs[:], lhsT=lmask, rhs=cnt[:, P - 1 : P], start=True, stop=True)
        offs = consts.tile([NP, 1], F32)
        nc.vector.tensor_copy(offs[:], poffs[:])

        # ---- E5 = exp(Q5 K5.T / 8) tiled to rows (b,a); block-diag e5blk ----
        ps5 = ps_small.tile([NP, V], F32, tag="sm")
        nc.tensor.matmul(ps5[:], lhsT=q5t[:], rhs=k5t[:], start=True, stop=True)
        e5r = consts.tile([NP, V], BF16)
        nc.scalar.activation(e5r[:], ps5[:], Exp, scale=0.125)
        e5blk = consts.tile([NP, NP], BF16)
        nc.vector.tensor_tensor(
            e5blk[:].rearrange("p (b t) -> p b t", t=V),
            bmask.rearrange("p (b t) -> p b t", t=V),
            e5r[:, None, :].to_broadcast((NP, KB, V)),
            mult,
        )

        # ---- G = E5[x_q, :] via block-diag matmul; W = (cnt + offs) * G ----
        ppg = ps_pg.tile([NP, P], F32)
        nc.tensor.matmul(ppg[:], lhsT=e5blk[:], rhs=oh[:], start=True, stop=True)
        w = consts.tile([NP, P], BF16)
        nc.vector.scalar_tensor_tensor(w[:], cnt[:], offs[:], ppg[:], add, mult)

        # ---- output: 4 blocks per wide matmul, normalize, DMA ----
        G4 = min(4, KB)
        for g in range(KB // G4):
            po4 = ps_o.tile([P, G4, D + 1], F32, tag="o")
            nc.tensor.matmul(
                po4[:], lhsT=w[:], rhs=rsel[:, g * G4 : (g + 1) * G4, :],
                start=True, stop=True,
            )
            rc = outp.tile([P, G4], F32, tag="rc")
            nc.vector.reciprocal(rc[:], po4[:, :, D : D + 1].rearrange("p b o -> p (b o)"))
            rs = outp.tile([P, G4, D], F32, tag="rs")
            nc.vector.tensor_tensor(
                rs[:], po4[:, :, 0:D],
                rc[:, :, None].to_broadcast((P, G4, D)),
                mult,
            )
            nc.sync.dma_start(
                out[g * G4 * P : (g + 1) * G4 * P, :].rearrange("(b p) d -> p b d", p=P),
                rs[:],
            )


def build_nc(S=S, mode=None):
    # Bacc (not plain Bass): its compile() pass splits multi-waits off
    # matmuls — TRN2 fp32 self-loading matmuls only encode one wait
    nc = bacc.Bacc(trn_type="TRN2", target_bir_lowering=False, debug=False)
    aps = {}
    aps["x"] = nc.dram_tensor("x", [_np(S), P], I32, kind="ExternalInput").ap()
    aps["cb"] = nc.dram_tensor(
        "cb", [max(D + 1, _np(S)), _cbw(S)], F32, kind="ExternalInput"
    ).ap()
    aps["out"] = nc.dram_tensor("out", [S, D], F32, kind="ExternalOutput").ap()
    with tile.TileContext(nc) as tc:
        _body(tc, aps, S=S)
    nc.compile()
    return nc


def make_in_maps(x, emb_table, wq, bq, wk, bk, wv, bv, S=S, n_cores=N_CORES):
    x = np.asarray(x).astype(np.int32)
    emb_table = np.asarray(emb_table, dtype=np.float32)
    NP = _np(S)
    KB = S // P
    CBP = max(D + 1, NP)

    def aug(wt, bias):
        return np.vstack(
            [np.asarray(wt, np.float32).T, np.asarray(bias, np.float32)[None, :]]
        )  # [D+1, D]

    cbuf = np.zeros((CBP, _cbw(S)), np.float32)
    cbuf[: D + 1, 0:V] = np.vstack([emb_table.T, np.ones((1, V), np.float32)])
    cbuf[: D + 1, V : V + D] = aug(wq, bq)
    cbuf[: D + 1, V + D : V + 2 * D] = aug(wk, bk)
    cbuf[: D + 1, V + 2 * D : V + 3 * D] = aug(wv, bv)
    cbuf[D, V + 3 * D] = 1.0  # e_D column -> ones column of V5aug
    c0 = V + 3 * D + 1
    pid = np.arange(NP)
    cbuf[:NP, c0] = pid % V  # token id per packed partition
    same_t = pid[:, None] % V == pid[None, :] % V
    cbuf[:NP, c0 + 1 : c0 + 1 + NP] = same_t & (
        pid[:, None] // V < pid[None, :] // V
    )
    cbuf[:NP, c0 + 1 + NP : c0 + 1 + 2 * NP] = (
        pid[:, None] // V == pid[None, :] // V
    )
    cbuf[:NP, c0 + 1 + 2 * NP : c0 + 1 + 2 * NP + KB] = (
        pid[:, None] // V == np.arange(KB)[None, :]
    )
    cbuf = np.ascontiguousarray(cbuf)

    def pack_x(xc):
        # [NP, P]: partition (b, t) = b*V + t holds x[b*128 : (b+1)*128]
        blocks = xc[:S].reshape(KB, 1, P)
        return np.ascontiguousarray(
            np.broadcast_to(blocks, (KB, V, P)).reshape(NP, P)
        )

    return [dict(x=pack_x(x[c]), cb=cbuf) for c in range(n_cores)]


_NC_CACHE = {}


def _get_nc(S=S):
    if S not in _NC_CACHE:
        _NC_CACHE[S] = build_nc(S=S)
    return _NC_CACHE[S]


def run(inputs, trace=False, **kw):
    in_maps = make_in_maps(**inputs)
    nc = _get_nc()
    res = run_bass_kernel_spmd(nc, in_maps, core_ids=list(range(N_CORES)), trace=trace, **kw)
    out = np.stack([res.results[c]["out"] for c in range(N_CORES)])
    return out, res


def kernel(x, emb_table, wq, bq, wk, bk, wv, bv):
    out, _ = run(dict(x=x, emb_table=emb_table, wq=wq, bq=bq, wk=wk, bk=bk,
                      wv=wv, bv=bv))
    return out


# revision 7
# speedup vs baseline: 3.2150x; 1.1026x over previous
"""Trainium2 Bass kernel for a single-layer causal-attention decoder.

Key algebraic shortcut: VOCAB=5 and the model has no positional encoding,
so Q[q], K[k], V[k] depend only on the token ids x_q, x_k. The [S, S]
attention therefore collapses to prefix token counts:

    E5[a, t]  = exp(Q5[a] . K5[t] / 8)          # [5, 5]
    C[t, q]   = #{k <= q : x_k = t}             # prefix counts
    out[q, :] = sum_t C[t,q] E5[x_q,t] V5[t,:] / sum_t C[t,q] E5[x_q,t]

which is O(S*V) work instead of O(S^2*D). No S x S matrices exist at all.

Device layout (per core; data-parallel over batch, one sequence per core):
  - positions packed 16 blocks x 128 on partitions: partition (b, t) = b*5+t
    holds token t's lane for position block b -> all elementwise work is
    [80, 128] instead of [5, 2048].
  - one-hot via is_equal(x_broadcast, t-column); in-block inclusive prefix
    count via the DVE tensor_tensor_scan; cross-block carries via one tiny
    matmul with a host-packed [80, 80] lower-block mask, folded into the
    weight multiply with scalar_tensor_tensor.
  - E5 selection by token is an [80, 80]-block-diag matmul (e5blk built
    with one masked multiply); W = C * E5[x_q, :].
  - output: po [128, 4, 65] = W.T @ R_g where R[(b,t), (bb,m)] =
    [b == bb] * V5aug[t, m] (built once on gpsimd) — 4 wide matmuls cover
    all 16 position blocks; the ones column of V5aug gives the softmax
    denominator; reciprocal + multiply + one DMA per group.
Softmax max-subtraction is skipped; |scores/8| < ~2 so exp is tame and the
normalization cancels it exactly.
"""

import numpy as np

import concourse.bass as bass
import concourse.mybir as mybir
import concourse.tile as tile
from concourse import bacc
from concourse.bass_utils import run_bass_kernel_spmd

F32 = mybir.dt.float32
BF16 = mybir.dt.bfloat16
I32 = mybir.dt.int32

B = 8
S = 2048
D = 64
V = 5
P = 128
N_CORES = 8
MODE = "bf16"


def _np(S):
    return V * (S // P)  # packed partitions: (block b, token t) -> b*V + t


def _cbw(S):
    # etT | wq | wk | wv_aug | io | Lmask | blkmask | hmask
    return V + 3 * D + 1 + 1 + 2 * _np(S) + (S // P)


def _body(tc, aps, S):
    nc = tc.nc
    x, cb, out = aps["x"], aps["cb"], aps["out"]
    KB = S // P          # position blocks (= 16 at S=2048)
    NP = V * KB          # packed partitions (= 80)
    CBP = max(D + 1, NP)
    Exp = mybir.ActivationFunctionType.Exp
    add = mybir.AluOpType.add
    mult = mybir.AluOpType.mult
    bypass = mybir.AluOpType.bypass
    is_equal = mybir.AluOpType.is_equal

    from contextlib import ExitStack
    with ExitStack() as ctx:
        consts = ctx.enter_context(tc.tile_pool(name="consts", bufs=1))
        outp = ctx.enter_context(tc.tile_pool(name="outp", bufs=4))
        ps_small = ctx.enter_context(tc.tile_pool(name="ps_small", bufs=2, space="PSUM"))
        ps_pg = ctx.enter_context(tc.tile_pool(name="ps_pg", bufs=1, space="PSUM"))
        ps_o = ctx.enter_context(tc.tile_pool(name="ps_o", bufs=4, space="PSUM"))

        # ---- constants in ----
        c0 = V + 3 * D + 1
        cb_sb = consts.tile([CBP, _cbw(S)], F32)
        nc.sync.dma_start(cb_sb[:], cb[:])
        etT = cb_sb[0 : D + 1, 0:V]                      # emb.T + ones row
        wqa = cb_sb[0 : D + 1, V : V + D]
        wka = cb_sb[0 : D + 1, V + D : V + 2 * D]
        wva = cb_sb[0 : D + 1, V + 2 * D : V + 3 * D + 1]
        io = cb_sb[0:NP, c0 : c0 + 1]                    # t = p % V
        lmask = cb_sb[0:NP, c0 + 1 : c0 + 1 + NP]
        bmask = cb_sb[0:NP, c0 + 1 + NP : c0 + 1 + 2 * NP]
        hmask = cb_sb[0:NP, c0 + 1 + 2 * NP : c0 + 1 + 2 * NP + KB]

        # ---- x packed+replicated on host: partition (b, t) holds x[b*128:(b+1)*128] ----
        xb = consts.tile([NP, P], I32)
        nc.sync.dma_start(xb[:], x[:])

        # ---- one-hot + in-block prefix counts first: DVE never stalls on PE ----
        oh = consts.tile([NP, P], BF16)
        nc.vector.tensor_tensor(oh[:], xb[:], io.to_broadcast((NP, P)), is_equal)
        cnt = consts.tile([NP, P], F32)
        nc.vector.tensor_tensor_scan(cnt[:], oh[:], oh[:], 0.0, add, bypass)

        # ---- tiny projected tables ----
        # V5aug80 first: it gates the gpsimd rsel chain
        etT80 = consts.tile([D + 1, NP], F32)
        nc.vector.tensor_copy(
            etT80[:].rearrange("d (b t) -> d b t", t=V),
            etT[:, None, :].to_broadcast((D + 1, KB, V)),
        )
        pvv = ps_small.tile([NP, D + 1], F32, tag="sm")
        nc.tensor.matmul(pvv[:], lhsT=etT80[:], rhs=wva, start=True, stop=True)
        v5aug = consts.tile([NP, D + 1], BF16)
        nc.scalar.copy(v5aug[:], pvv[:])  # ACT reads PSUM; gpsimd cannot

        # R [(b,t), (bb, m)] = [b == bb] * V5aug[t, m], one gpsimd chunk per
        # output group so group g's matmul never waits on later chunks
        G4 = min(4, KB)
        NG = KB // G4
        rsel = consts.tile([NP, KB, D + 1], BF16)
        for g in range(NG):
            gs = slice(g * G4, (g + 1) * G4)
            nc.gpsimd.tensor_tensor(
                rsel[:, gs, :],
                hmask[:, gs, None].to_broadcast((NP, G4, D + 1)),
                v5aug[:, None, :].to_broadcast((NP, G4, D + 1)),
                mult,
            )

        # Q5T tiled KB times along columns: [D, NP], col (b,t) = Q5T[:, t]
        pq = ps_small.tile([D, V], F32, tag="sm")
        nc.tensor.matmul(pq[:], lhsT=wqa, rhs=etT, start=True, stop=True)
        q5t = consts.tile([D, NP], F32)
        nc.vector.tensor_copy(
            q5t[:].rearrange("d (b t) -> d b t", t=V),
            pq[:, None, :].to_broadcast((D, KB, V)),
        )
        pk = ps_small.tile([D, V], F32, tag="sm")
        nc.tensor.matmul(pk[:], lhsT=wka, rhs=etT, start=True, stop=True)
        k5t = consts.tile([D, V], F32)
        nc.scalar.copy(k5t[:], pk[:])  # ACT is idle; keeps DVE free

        # ---- E5 = exp(Q5 K5.T / 8) tiled to rows (b,a); block-diag e5blk ----
        ps5 = ps_small.tile([NP, V], F32, tag="sm")
        nc.tensor.matmul(ps5[:], lhsT=q5t[:], rhs=k5t[:], start=True, stop=True)
        e5r = consts.tile([NP, V], BF16)
        nc.scalar.activation(e5r[:], ps5[:], Exp, scale=0.125)

        # ---- cross-block carries: offs[(b,t)] = sum_{b'<b} total[(b',t)] ----
        poffs = ps_small.tile([NP, 1], F32, tag="sm")
        nc.tensor.matmul(poffs[:], lhsT=lmask, rhs=cnt[:, P - 1 : P], start=True, stop=True)
        offs = consts.tile([NP, 1], F32)
        nc.vector.tensor_copy(offs[:], poffs[:])
        e5blk = consts.tile([NP, NP], BF16)
        nc.vector.tensor_tensor(
            e5blk[:].rearrange("p (b t) -> p b t", t=V),
            bmask.rearrange("p (b t) -> p b t", t=V),
            e5r[:, None, :].to_broadcast((NP, KB, V)),
            mult,
        )

        # ---- G = E5[x_q, :] via block-diag matmul; W = (cnt + offs) * G ----
        ppg = ps_pg.tile([NP, P], F32)
        nc.tensor.matmul(ppg[:], lhsT=e5blk[:], rhs=oh[:], start=True, stop=True)
        w = consts.tile([NP, P], BF16)
        nc.vector.scalar_tensor_tensor(w[:], cnt[:], offs[:], ppg[:], add, mult)

        # ---- output: 4 blocks per wide matmul, normalize, DMA ----
        # out DRAM is [P, KB, D] p-major: partition p writes one contiguous
        # 1KB segment per group (host transposes back on unshard)
        for g in range(NG):
            po4 = ps_o.tile([P, G4, D + 1], F32, tag="o")
            nc.tensor.matmul(
                po4[:], lhsT=w[:], rhs=rsel[:, g * G4 : (g + 1) * G4, :],
                start=True, stop=True,
            )
            rc = outp.tile([P, G4], F32, tag="rc")
            nc.vector.reciprocal(rc[:], po4[:, :, D : D + 1].rearrange("p b o -> p (b o)"))
            rs = outp.tile([P, G4, D], F32, tag="rs")
            nc.vector.tensor_tensor(
                rs[:], po4[:, :, 0:D],
                rc[:, :, None].to_broadcast((P, G4, D)),
                mult,
            )
            nc.sync.dma_start(out[:, g * G4 : (g + 1) * G4, :], rs[:])


def build_nc(S=S, mode=None):
    # Bacc (not plain Bass): its compile() pass splits multi-waits off
    # matmuls — TRN2 fp32 self-loading matmuls only encode one wait
    nc = bacc.Bacc(trn_type="TRN2", target_bir_lowering=False, debug=False)
    aps = {}
    aps["x"] = nc.dram_tensor("x", [_np(S), P], I32, kind="ExternalInput").ap()
    aps["cb"] = nc.dram_tensor(
        "cb", [max(D + 1, _np(S)), _cbw(S)], F32, kind="ExternalInput"
    ).ap()
    aps["out"] = nc.dram_tensor("out", [P, S // P, D], F32, kind="ExternalOutput").ap()
    with tile.TileContext(nc) as tc:
        _body(tc, aps, S=S)
    nc.compile()
    return nc


def make_in_maps(x, emb_table, wq, bq, wk, bk, wv, bv, S=S, n_cores=N_CORES):
    x = np.asarray(x).astype(np.int32)
    emb_table = np.asarray(emb_table, dtype=np.float32)
    NP = _np(S)
    KB = S // P
    CBP = max(D + 1, NP)

    def aug(wt, bias):
        return np.vstack(
            [np.asarray(wt, np.float32).T, np.asarray(bias, np.float32)[None, :]]
        )  # [D+1, D]

    cbuf = np.zeros((CBP, _cbw(S)), np.float32)
    cbuf[: D + 1, 0:V] = np.vstack([emb_table.T, np.ones((1, V), np.float32)])
    cbuf[: D + 1, V : V + D] = aug(wq, bq)
    cbuf[: D + 1, V + D : V + 2 * D] = aug(wk, bk)
    cbuf[: D + 1, V + 2 * D : V + 3 * D] = aug(wv, bv)
    cbuf[D, V + 3 * D] = 1.0  # e_D column -> ones column of V5aug
    c0 = V + 3 * D + 1
    pid = np.arange(NP)
    cbuf[:NP, c0] = pid % V  # token id per packed partition
    same_t = pid[:, None] % V == pid[None, :] % V
    cbuf[:NP, c0 + 1 : c0 + 1 + NP] = same_t & (
        pid[:, None] // V < pid[None, :] // V
    )
    cbuf[:NP, c0 + 1 + NP : c0 + 1 + 2 * NP] = (
        pid[:, None] // V == pid[None, :] // V
    )
    cbuf[:NP, c0 + 1 + 2 * NP : c0 + 1 + 2 * NP + KB] = (
        pid[:, None] // V == np.arange(KB)[None, :]
    )
    cbuf = np.ascontiguousarray(cbuf)

    def pack_x(xc):
        # [NP, P]: partition (b, t) = b*V + t holds x[b*128 : (b+1)*128]
        blocks = xc[:S].reshape(KB, 1, P)
        return np.ascontiguousarray(
            np.broadcast_to(blocks, (KB, V, P)).reshape(NP, P)
        )

    return [dict(x=pack_x(x[c]), cb=cbuf) for c in range(n_cores)]


_NC_CACHE = {}


def _get_nc(S=S):
    if S not in _NC_CACHE:
        _NC_CACHE[S] = build_nc(S=S)
    return _NC_CACHE[S]


def run(inputs, trace=False, **kw):
    in_maps = make_in_maps(**inputs)
    nc = _get_nc()
    res = run_bass_kernel_spmd(nc, in_maps, core_ids=list(range(N_CORES)), trace=trace, **kw)
    # device keeps [P, KB, D] (one contiguous segment per partition per DMA);
    # unshard transposes back to [S, D]
    out = np.stack([
        np.ascontiguousarray(
            res.results[c]["out"].transpose(1, 0, 2).reshape(S, D)
        )
        for c in range(N_CORES)
    ])
    return out, res


def kernel(x, emb_table, wq, bq, wk, bk, wv, bv):
    out, _ = run(dict(x=x, emb_table=emb_table, wq=wq, bq=bq, wk=wk, bk=bk,
                      wv=wv, bv=bv))
    return out
